# revision 1
# baseline (speedup 1.0000x reference)
"""Trainium2 Bass kernel for a 2-layer LSTM (B=131072, T=49, H=8) + linear head.

Sharding: pure data parallel over batch across 8 cores (BC = 16384 each).

Layout: BOTH layers stacked on the partition dim: partition p = 16*g + j for
8 batch groups g; j in 0..7 holds layer-0 state dims, j in 8..15 layer-1.
Layer 1 runs one timestep behind layer 0, so the state tile [h1_s; h2_{s-1}]
lets ONE K=128 matmul per gate produce layer-0 gates for step s+1 AND layer-1
gates for step s simultaneously. Batch per core = 8 groups x 2048 columns,
processed as 4 pipelined chunks of N=512.

Per step per chunk: 4 serial K=128 gate matmuls (block-diagonal stacked
weights) + two concurrent K=8 row-strip matmul groups at tile_position
(32q,0) (the x-term, and the biases against a static all-ones tile) into a
[128, 2048] PSUM tile laid out [i | f | o | 2g]. The g-gate is evaluated as
tanh(x) = 2*sigmoid(2x)-1 with the 2x folded into the weights, so ONE
Sigmoid ACTIVATE (FD=2048) covers all four gates. The cell state is kept as
c' = c/2, making the update c' = f*c' + i*(s-0.5) — one fused
scalar_tensor_tensor — while tanh(c) = tanh(2c') rides the ACT scale
operand for free. c' stays fp32; activations output fp32 for accuracy. All
elementwise work runs on the DVE (GpSimd shares the DVE's SBUF port, so
co-scheduling them is slower than the DVE alone); the four chunks form four
independent recurrence chains that pipeline across engines.

Step 0 uses an aux weight image whose layer-1 biases are -30, which drives
sigma to ~0 and exactly zero-initializes the skewed layer-1 state. Step 49
(epilogue) computes the final layer-1 step; its layer-0 output is unused.
"""

import numpy as np

B, T, H = 131072, 49, 8
NCORES = 8
BC = B // NCORES          # 16384 per core
G = 8                     # batch groups; partitions = G * 16 = 128
NCH = 4                   # chunks per core
N = 512                   # columns per chunk
S = T + 1                 # pipeline steps (layer 1 skewed by one)
SB = 13                   # steps per staged X block
NB = (S + SB - 1) // SB   # 4 blocks
SLOT_TO_REF = [0, 1, 3, 2]  # gate slot order [i, f, o, g]; ref row-blocks i,f,g,o

_PROGRAM_CACHE = {}

MM_DT = "float16"


def _build_program():
    import concourse.bacc as bacc
    import concourse.mybir as mybir
    import concourse.tile as tile

    f32 = mybir.dt.float32
    f16 = getattr(mybir.dt, MM_DT)
    AF = mybir.ActivationFunctionType
    ALU = mybir.AluOpType

    nc = bacc.Bacc("TRN2", target_bir_lowering=False, debug=False)

    xt_d = nc.dram_tensor("xt", [S, BC], f16, kind="ExternalInput")
    wg_d = nc.dram_tensor("wg", [4, 128, 128], f16, kind="ExternalInput")
    wax_d = nc.dram_tensor("wax", [128, 128], f16, kind="ExternalInput")
    wab_d = nc.dram_tensor("wab", [128, 128], f16, kind="ExternalInput")
    wab0_d = nc.dram_tensor("wab0", [128, 128], f16, kind="ExternalInput")
    whead_d = nc.dram_tensor("whead", [128, 8], f16, kind="ExternalInput")
    headb_d = nc.dram_tensor("headb", [8, 1], f32, kind="ExternalInput")
    y_d = nc.dram_tensor("y", [G, NCH, N], f32, kind="ExternalOutput")

    with tile.TileContext(nc) as tc:
        with (
            tc.tile_pool(name="w", bufs=1) as wpool,
            tc.tile_pool(name="state", bufs=1) as spool,
            tc.tile_pool(name="xs", bufs=2) as xpool,
            tc.tile_pool(name="work", bufs=2) as work,
        ):
            wax = wpool.tile([128, 128], f16, tag="wax")
            nc.sync.dma_start(wax[:], wax_d[:])
            wab0 = wpool.tile([128, 128], f16, tag="wab0")
            nc.sync.dma_start(wab0[:], wab0_d[:])
            wq = []
            for q in range(4):
                wt = wpool.tile([128, 128], f16, tag=f"wg{q}", name=f"wg{q}")
                nc.sync.dma_start(wt[:], wg_d[q])
                wq.append(wt)

            # State: h = [h1_s ; h2_{s-1}] fp16, c fp32. Cols = 512*ch + n.
            h_st = spool.tile([128, NCH * N], f16, tag="h")
            nc.gpsimd.memset(h_st[:], 0.0)
            c_st = spool.tile([128, NCH * N], f32, tag="c")
            nc.gpsimd.memset(c_st[:], 0.0)

            # X staging: rows 32q+g carry x for group g, replicated per
            # gate strip q; the K=8 x-matmul reads only those rows. Biases
            # ride a second K=8 matmul against the static all-ones tile.
            ones = wpool.tile([128, N], f16, tag="ones")
            nc.gpsimd.memset(ones[:], 1.0)

            def stage_x(ch, b, split_first=False):
                sbn = min(SB, S - b * SB)
                xt_blk = xpool.tile([128, SB * N], f16, tag=f"xs{ch}",
                                    name=f"xsblk{ch}_{b}")
                # src: element (g, s', n) at xt[b*SB+s', g*2048 + ch*512 + n]
                s0 = 1 if split_first else 0
                if split_first:
                    # step-0 slice rides its own small DMA so the first
                    # matmuls aren't gated by the full-block transfer
                    src0 = xt_d[b * SB, :].rearrange(
                        "(g c n) -> g c n", g=G, c=NCH)[:, ch]
                    for q in range(4):
                        nc.sync.dma_start(
                            xt_blk[32 * q : 32 * q + 8, 0:N], src0)
                src = xt_d[b * SB + s0 : b * SB + sbn, :].rearrange(
                    "s (g c n) -> g c s n", g=G, c=NCH
                )[:, ch]
                for q in range(4):
                    nc.sync.dma_start(
                        xt_blk[32 * q : 32 * q + 8, s0 * N : sbn * N].rearrange(
                            "p (s n) -> p s n", n=N
                        ),
                        src,
                    )
                return xt_blk

            xblk = {ch: stage_x(ch, 0, split_first=True) for ch in range(NCH)}
            wab = wpool.tile([128, 128], f16, tag="wab")
            nc.sync.dma_start(wab[:], wab_d[:])
            whead = wpool.tile([128, 8], f16, tag="whead")
            nc.sync.dma_start(whead[:], whead_d[:])
            headb = wpool.tile([8, 1], f32, tag="headb")
            nc.sync.dma_start(headb[:], headb_d[:])

            with tc.tile_pool(name="psum_gates", bufs=2, space="PSUM") as pg:
                for s in range(S):
                    b, toff = s // SB, (s % SB) * N
                    if s % SB == 0 and b + 1 < NB:
                        for ch in range(NCH):
                            xblk[ch, b + 1] = stage_x(ch, b + 1)
                    bimg = wab0 if s == 0 else wab
                    for ch in range(NCH):
                        cc = slice(ch * N, (ch + 1) * N)
                        gates = pg.tile([128, 4 * N], f32, tag="g")
                        for q in range(4):
                            nc.tensor.matmul(
                                gates[:, q * N : (q + 1) * N],
                                wq[q][:],
                                h_st[:, cc],
                                start=True,
                                stop=False,
                                skip_group_check=True,
                            )
                        xb = xblk[ch] if b == 0 else xblk[ch, b]
                        for q in range(4):
                            nc.tensor.matmul(
                                gates[:, q * N : (q + 1) * N],
                                wax[32 * q : 32 * q + 8, :],
                                xb[32 * q : 32 * q + 8, toff : toff + N],
                                start=False,
                                stop=False,
                                tile_position=(32 * q, 0),
                                skip_group_check=True,
                            )
                        for q in range(4):
                            nc.tensor.matmul(
                                gates[:, q * N : (q + 1) * N],
                                bimg[32 * q : 32 * q + 8, :],
                                ones[32 * q : 32 * q + 8, :],
                                start=False,
                                stop=True,
                                tile_position=(32 * q, 0),
                                skip_group_check=True,
                            )
                        # bufs=1: step s+1's writes of these tiles are already
                        # ordered after step s's reads via the h/c recurrence.
                        sif = work.tile([128, 4 * N], f32, tag=f"sif{ch}", bufs=1)
                        nc.scalar.activation(sif[:], gates[:], AF.Sigmoid)
                        # State is kept as c' = c/2, so with s = sigma(2g):
                        # c' = f*c' + i*(s - 0.5), and tanh(c) = tanh(2c')
                        # comes free via the ACT scale operand.
                        # GpSimd shares the SBUF port with the DVE: running it
                        # concurrently is slower than DVE alone, so the whole
                        # post-sigma chain stays on the DVE, per-chunk (four
                        # independent chains hide the recurrence latency).
                        m1 = work.tile([128, N], f32, tag=f"m1{ch}", bufs=1)
                        nc.vector.tensor_mul(m1[:], sif[:, N : 2 * N], c_st[:, cc])
                        m2 = work.tile([128, N], f32, tag=f"m2{ch}", bufs=1)
                        nc.vector.scalar_tensor_tensor(
                            m2[:], sif[:, 3 * N : 4 * N], 0.5, sif[:, 0:N],
                            ALU.subtract, ALU.mult,
                        )
                        nc.vector.tensor_add(c_st[:, cc], m1[:], m2[:])
                        th = work.tile([128, N], f16, tag=f"th{ch}", bufs=1)
                        nc.scalar.activation(th[:], c_st[:, cc], AF.Tanh,
                                             scale=2.0)
                        nc.vector.tensor_mul(
                            h_st[:, cc], sif[:, 2 * N : 3 * N], th[:]
                        )

                # Head: still inside the gates pool — the head matmuls
                # write an 8-partition strip of a normal "g"-tag PSUM tile,
                # so there is no pool-swap barrier.
                r2 = work.tile([128, NCH * N], f16, tag="r2", bufs=1)
                for ch in range(NCH):
                    # per-chunk relu: chunk ch's head starts as soon as its
                    # own final h-mul lands, not after the last chunk's
                    nc.scalar.activation(r2[:, ch * N : (ch + 1) * N],
                                         h_st[:, ch * N : (ch + 1) * N],
                                         AF.Relu)
                g2 = pg.tile([128, 4 * N], f32, tag="g")
                for ch in range(NCH):
                    nc.tensor.matmul(
                        g2[0:8, ch * N : (ch + 1) * N],
                        whead[:],
                        r2[:, ch * N : (ch + 1) * N],
                        start=True, stop=True, skip_group_check=True,
                    )
                ysb = work.tile([8, NCH * N], f32, tag="ysb")
                nc.scalar.activation(ysb[:], g2[0:8, :], AF.Relu, bias=headb[:])
                nc.sync.dma_start(
                    y_d[:], ysb[:].rearrange("p (c n) -> p c n", n=N))

    nc.compile()
    return nc


def _get_program():
    if "nc" not in _PROGRAM_CACHE:
        _PROGRAM_CACHE["nc"] = _build_program()
    return _PROGRAM_CACHE["nc"]


def _pack_weights(W_ih0, W_hh0, b_ih0, b_hh0, W_ih1, W_hh1, b_ih1, b_hh1, W_lin, b_lin):
    b0 = (b_ih0 + b_hh0).astype(np.float32)
    b1 = (b_ih1 + b_hh1).astype(np.float32)
    wg = np.zeros((4, 128, 128), np.float32)
    wax = np.zeros((128, 128), np.float32)
    wab = np.zeros((128, 128), np.float32)
    whead = np.zeros((128, 8), np.float32)
    for q in range(4):
        r = SLOT_TO_REF[q]
        sc = 2.0 if q == 3 else 1.0
        hh0 = W_hh0[8 * r : 8 * r + 8, :] * sc   # [out j, in a]
        ih1 = W_ih1[8 * r : 8 * r + 8, :] * sc
        hh1 = W_hh1[8 * r : 8 * r + 8, :] * sc
        for g in range(G):
            rows_l0 = slice(16 * g, 16 * g + 8)       # input dims a (h1)
            rows_l1 = slice(16 * g + 8, 16 * g + 16)  # input dims a (h2)
            cols_l0 = slice(16 * g, 16 * g + 8)       # output dims j (layer0)
            cols_l1 = slice(16 * g + 8, 16 * g + 16)  # output dims j (layer1)
            wg[q][rows_l0, cols_l0] = hh0.T           # lhsT[a, j]
            wg[q][rows_l0, cols_l1] = ih1.T
            wg[q][rows_l1, cols_l1] = hh1.T
            wax[32 * q + g, cols_l0] = W_ih0[8 * r : 8 * r + 8, 0] * sc
            wab[32 * q + g, cols_l0] = b0[8 * r : 8 * r + 8] * sc
            wab[32 * q + g, cols_l1] = b1[8 * r : 8 * r + 8] * sc
    wab0 = wab.copy()
    for q in range(4):
        for g in range(G):
            wab0[32 * q + g, 16 * g + 8 : 16 * g + 16] = -30.0
    for g in range(G):
        whead[16 * g + 8 : 16 * g + 16, g] = W_lin[0, :]
    headb = np.full((8, 1), np.float32(b_lin[0]), np.float32)
    mmdt = np.dtype(MM_DT)
    return {
        "wg": wg.astype(mmdt),
        "wax": wax.astype(mmdt),
        "wab": wab.astype(mmdt),
        "wab0": wab0.astype(mmdt),
        "whead": whead.astype(mmdt),
        "headb": headb,
    }


def _make_in_maps(X, packs):
    Xt = np.asarray(X)[:, :, 0].T.astype(np.dtype(MM_DT))  # [T, B]
    in_maps = []
    for r in range(NCORES):
        blk = Xt[:, r * BC : (r + 1) * BC]
        xtp = np.concatenate([blk, blk[T - 1 : T]], axis=0)  # [S, BC]
        m = dict(packs)
        m["xt"] = np.ascontiguousarray(xtp)
        in_maps.append(m)
    return in_maps


def kernel(X, W_ih0, W_hh0, b_ih0, b_hh0, W_ih1, W_hh1, b_ih1, b_hh1, W_lin, b_lin,
           _trace=False, _trace_kwargs=None):
    from concourse.bass_utils import run_bass_kernel_spmd

    packs = _pack_weights(
        np.asarray(W_ih0), np.asarray(W_hh0), np.asarray(b_ih0), np.asarray(b_hh0),
        np.asarray(W_ih1), np.asarray(W_hh1), np.asarray(b_ih1), np.asarray(b_hh1),
        np.asarray(W_lin), np.asarray(b_lin),
    )
    nc = _get_program()
    in_maps = _make_in_maps(X, packs)
    res = run_bass_kernel_spmd(
        nc, in_maps, list(range(NCORES)), trace=_trace, **(_trace_kwargs or {})
    )
    y = np.concatenate(
        [res.results[r]["y"].reshape(BC) for r in range(NCORES)]
    )
    if _trace:
        return y, res
    return y

